# revision 45
# baseline (speedup 1.0000x reference)
"""Trainium2 Bass kernel for nn_ConcentrationLoss (raw-Bass SPMD version).

Math per (b, c) slice of pred/target [B,C,H,W]:
    mass = sum(t); cy = sum(t*y)/mass; cx = sum(t*x)/mass
    per_slice = mean(sigmoid(pred) * ((y-cy)^2 + (x-cx)^2))
    loss = mean(per_slice over slices with mass > 0)

Streaming moment sums per slice (centered coords y' = y-(H-1)/2,
x' = x-(W-1)/2):
    T0, Ty'        from target      S0, Sy', Sy'y'  from s = sigmoid(pred)
    Tx', Sx', Sx'x' (x-moments)

Host packs target+pred into one fp8-e4m3 tensor, two slices per 1 MiB DMA
([128, 8192]: slice j of the pair at cols j*4096, target chunks then pred
chunks, 8 KB contiguous per partition) — quarter the fp32 DMA traffic.
Quantization noise averages out over the 512x512 means (measured rel err
~2e-4 in fp64 simulation of this exact pipeline; gate is 2e-2).

On device: ScalarE sigmoids the pred half (fp8 in -> bf16 out); TensorE
contracts the h/partition axis — target matmuls in fp8 against y'/32-scaled
fp8 weight columns (descaled on host), pred matmuls in bf16 — accumulating
each slice's 5 moment rows [t, s, t*y'/32, s*y', s*y'^2] into one of 8 PSUM
banks. VectorE copies each bank to a base-0 scratch, GPSIMD SWDGE-DMAs the
scratch into a [100, 512] staging tile at partition offset 5*slice (DVE
cannot write at unaligned partition offsets), and VectorE runs one batched
stage2 (row sums, x'- and x'^2-weighted sums) per pass. Data-parallel on
batch across 8 cores; the scalar tail (centroid divides + masked mean) is
combined on the host.

Raw-Bass engine blocks with standalone wait_ge instructions (the toolchain
rejects instructions carrying more than one attached semaphore wait, which
rules out the Tile scheduler).
"""

import sys

for _p in ("/opt/trn_rl_repo",):
    if _p not in sys.path:
        sys.path.append(_p)

import numpy as np
import ml_dtypes

import concourse.bass as bass
from concourse import mybir
from concourse.bass_utils import run_bass_kernel_spmd

B, C, H, W = 16, 10, 512, 512
NCORES = 8
BPC = B // NCORES          # batches per core
S = BPC * C                # slices per core (20)
DPAIR = S // 2             # DMA pairs per core (2 slices / 1 MiB DMA)
NCHUNK = H // 128          # 4 h-chunks per slice
FW = NCHUNK * W            # 2048: free size of one packed half-tile
HW = float(H * W)
NROW = 5 * S               # 100 staging rows across all slices
YSCALE = 32.0              # target y' weight scale (power of 2, fp8-exact)

_CACHE = {}


def _moment_weights():
    """Returns (wmt fp8 [128, NCHUNK*5], wmp bf16 [128, NCHUNK*5]).
    Per chunk k: target lhsT cols k*5+0..4 -> psum rows [t, 0, t*y'/32, 0, 0];
    pred lhsT cols k*5+0..4 -> psum rows [0, s, 0, s*y', s*y'^2]."""
    wt = np.zeros((128, NCHUNK * 5), dtype=np.float64)
    wp = np.zeros((128, NCHUNK * 5), dtype=np.float64)
    for k in range(NCHUNK):
        yp = (np.arange(128, dtype=np.float64) + 128 * k) - (H - 1) / 2.0
        wt[:, k * 5 + 0] = 1.0
        wt[:, k * 5 + 2] = yp / YSCALE
        wp[:, k * 5 + 1] = 1.0
        wp[:, k * 5 + 3] = yp
        wp[:, k * 5 + 4] = yp * yp
    return wt.astype(ml_dtypes.float8_e4m3), wp.astype(ml_dtypes.bfloat16)


def _build_nc(repeat=1, mode="full"):
    """repeat>1 re-runs the full pipeline (slice index s % S) for timing
    benchmarks — output is overwritten with identical values each pass.
    mode: 'full' | 'nostage2' (skip vector/gpsimd) | 'nomm' (dma+sigmoid
    only) | 'dmaonly' — timing probes with garbage output."""
    R = repeat
    do_sig = mode in ("full", "nostage2", "nomm")
    do_mm = mode in ("full", "nostage2")
    do_st2 = mode == "full"
    nc = bass.Bass("TRN2", target_bir_lowering=False, debug=False)
    f32, bf16, f8 = mybir.dt.float32, mybir.dt.bfloat16, mybir.dt.float8e4

    x_d = nc.dram_tensor("x", [DPAIR, 128, 4 * FW], f8, kind="ExternalInput")
    wmt_d = nc.dram_tensor("wmt", [128, NCHUNK * 5], f8, kind="ExternalInput")
    wmp_d = nc.dram_tensor("wmp", [128, NCHUNK * 5], bf16, kind="ExternalInput")
    xcb_d = nc.dram_tensor("xcb", [NROW, W], f32, kind="ExternalInput")
    xc2b_d = nc.dram_tensor("xc2b", [NROW, W], f32, kind="ExternalInput")
    out_d = nc.dram_tensor("moments", [NROW, 4], f32, kind="ExternalOutput")

    NB = 4                               # xb ring depth (DMA pairs in flight)
    PB = 4                               # pb ring depth (one tile per pair)
    xb = [nc.alloc_sbuf_tensor(f"xb{b}", [128, 4 * FW], f8) for b in range(NB)]
    pb = [nc.alloc_sbuf_tensor(f"pb{b}", [128, 2 * FW], bf16) for b in range(PB)]
    wtsb = nc.alloc_sbuf_tensor("wtsb", [128, NCHUNK * 5], f8)
    wpsb = nc.alloc_sbuf_tensor("wpsb", [128, NCHUNK * 5], bf16)
    xcsb = nc.alloc_sbuf_tensor("xcsb", [NROW, W], f32)
    xc2sb = nc.alloc_sbuf_tensor("xc2sb", [NROW, W], f32)
    sc = [nc.alloc_sbuf_tensor(f"sc{b}", [5, W], f32) for b in range(4)]
    stg = nc.alloc_sbuf_tensor("stg", [NROW, W], f32)
    t1 = nc.alloc_sbuf_tensor("t1", [NROW, W], f32)
    t2 = nc.alloc_sbuf_tensor("t2", [NROW, W], f32)
    O = nc.alloc_sbuf_tensor("O", [NROW, 4], f32)
    ps = [nc.alloc_psum_tensor(f"ps{b}", [5, W], f32) for b in range(8)]

    csem = nc.alloc_semaphore("csem")    # const DMAs
    xdma = [nc.alloc_semaphore(f"xdma{b}") for b in range(NB)]  # pair DMAs by slot
    asem = nc.alloc_semaphore("asem")    # sigmoid done (1 per slice)
    pe = nc.alloc_semaphore("pe")        # matmul group done (1 per slice)
    vcp = nc.alloc_semaphore("vcp")      # psum->scratch copy done (1 per slice)
    gsem = nc.alloc_semaphore("gsem")    # scratch->stg dma done (16 per slice)
    dst2 = nc.alloc_semaphore("dst2")    # stage2 done (1 per pass)
    osem = nc.alloc_semaphore("osem")    # out DMA

    def targ_ap(b, s, k):
        return xb[b][:, (s % 2) * 2 * FW + k * W : (s % 2) * 2 * FW + (k + 1) * W]

    def pred_ap(b, s):
        return xb[b][:, (s % 2) * 2 * FW + FW : (s % 2) * 2 * FW + 2 * FW]

    with nc.Block() as block:

        @block.sync
        def _(sync):
            # first payload pair leads; consts follow back-to-back (same
            # queue => FIFO completion). PE needs wmt/wmp only after the
            # first sigmoid (~6us in); DVE needs xcb/xc2b only at stage2.
            sync.dma_start(xb[0][:], x_d[0]).then_inc(xdma[0], 16)
            sync.dma_start(wtsb[:], wmt_d[:]).then_inc(csem, 16)
            sync.dma_start(wpsb[:], wmp_d[:]).then_inc(csem, 16)
            sync.dma_start(xcsb[:], xcb_d[:]).then_inc(csem, 16)
            sync.dma_start(xc2sb[:], xc2b_d[:]).then_inc(csem, 16)
            for d in range(1, R * DPAIR):
                b = d % NB
                if d >= NB:
                    # xb[b] consumed by both slices of pair d-NB
                    if do_mm:
                        sync.wait_ge(pe, 2 * (d - NB) + 2)
                    elif do_sig:
                        sync.wait_ge(asem, d - NB + 1)
                    sync.wait_ge(xdma[b], 16 * (d // NB))
                sync.dma_start(xb[b][:], x_d[d % DPAIR]).then_inc(xdma[b], 16)
            if do_st2:
                sync.wait_ge(dst2, R)
            elif do_mm:
                sync.wait_ge(pe, 2 * R * DPAIR)
            elif do_sig:
                sync.wait_ge(asem, R * DPAIR)
            else:
                for b in range(NB):
                    n_b = (R * DPAIR - b + NB - 1) // NB
                    if n_b > 0:
                        sync.wait_ge(xdma[b], 16 * n_b)
            sync.dma_start(out_d[:], O[:]).then_inc(osem, 16)
            sync.wait_ge(osem, 16)

        if do_sig:

            @block.scalar
            def _(scalar):
                for d in range(R * DPAIR):
                    b = d % NB
                    scalar.wait_ge(xdma[b], 16 * (d // NB + 1))
                    if d >= PB and do_mm:
                        scalar.wait_ge(pe, 2 * (d - PB) + 2)  # pb[d%PB] consumed
                    # both pred halves of the pair in one ACT instruction
                    # (3D in-AP strides over the two slice blocks)
                    scalar.activation(
                        pb[d % PB][:].rearrange("p (j f) -> p j f", j=2),
                        xb[b][:].rearrange("p (j f) -> p j f", j=2)[:, :, FW : 2 * FW],
                        mybir.ActivationFunctionType.Sigmoid,
                    ).then_inc(asem, 1)
                    if do_st2 and d == R * DPAIR - 1:
                        # last slice's staging copy via low-latency HWDGE
                        # instead of gpsimd SWDGE — shortens the tail
                        scalar.wait_ge(vcp, R * S)
                        scalar.dma_start(
                            stg[5 * (S - 1) : 5 * S, :], sc[(S - 1) % 4][:]
                        ).then_inc(gsem, 16)

        if do_mm:

            @block.tensor
            def _(tensor):
                tensor.wait_ge(csem, 32)          # wmt+wmp loaded (first consts)
                for s in range(2 * R * DPAIR):
                    d = s // 2
                    b = d % NB
                    p = ps[s % 8]
                    if s % 2 == 0:
                        tensor.wait_ge(xdma[b], 16 * (d // NB + 1))
                    if s >= 8 and do_st2:
                        tensor.wait_ge(vcp, s - 7)  # psum bank copied (slice s-8)
                    for k in range(NCHUNK):
                        tensor.matmul(
                            p[:],
                            wtsb[:, k * 5 : k * 5 + 5],
                            targ_ap(b, s, k),
                            start=(k == 0),
                            stop=False,
                        )
                    tensor.wait_ge(asem, d + 1)   # pb ready for pred half
                    for k in range(NCHUNK):
                        mm = tensor.matmul(
                            p[:],
                            wpsb[:, k * 5 : k * 5 + 5],
                            pb[d % PB][:, (s % 2) * FW + k * W : (s % 2) * FW + (k + 1) * W],
                            start=False,
                            stop=(k == NCHUNK - 1),
                        )
                    mm.then_inc(pe, 1)

        if do_st2:

            @block.vector
            def _(vector):
                vector.wait_ge(csem, 64)
                for r in range(R):
                    for c in range(S):
                        s = r * S + c
                        vector.wait_ge(pe, s + 1)
                        if s >= 4:
                            vector.wait_ge(gsem, 16 * (s - 3))  # sc[s%4] drained
                        vector.tensor_copy(sc[s % 4][:], ps[s % 8][:]).then_inc(vcp, 1)
                    vector.wait_ge(gsem, 16 * S * (r + 1))      # stg fully packed
                    vector.reduce_sum(O[:, 0:1], stg[:], axis=mybir.AxisListType.X)
                    vector.tensor_mul(t1[:], stg[:], xcsb[:])
                    vector.reduce_sum(O[:, 1:2], t1[:], axis=mybir.AxisListType.X)
                    vector.tensor_mul(t2[:], stg[:], xc2sb[:])
                    vector.reduce_sum(
                        O[:, 2:3], t2[:], axis=mybir.AxisListType.X
                    ).then_inc(dst2, 1)

            @block.gpsimd
            def _(gpsimd):
                for s in range(R * S):
                    if s == R * S - 1:
                        continue          # final slice staged by scalar HWDGE
                    c = s % S
                    gpsimd.wait_ge(vcp, s + 1)
                    gpsimd.dma_start(stg[5 * c : 5 * c + 5, :], sc[s % 4][:]).then_inc(
                        gsem, 16
                    )

    return nc


def _host_inputs():
    xp = (np.arange(W, dtype=np.float64) - (W - 1) / 2.0).astype(np.float32)
    xcb = np.broadcast_to(xp, (NROW, W)).copy()
    xc2b = np.broadcast_to((xp * xp).astype(np.float32), (NROW, W)).copy()
    wmt, wmp = _moment_weights()
    return wmt, wmp, xcb, xc2b


def _get_built():
    if "nc" not in _CACHE:
        _CACHE["nc"] = _build_nc()
        _CACHE["consts"] = _host_inputs()
    return _CACHE["nc"], _CACHE["consts"]


def _pack_inputs(pred, target):
    """[B,C,H,W] fp32 pair -> per-core packed fp8 [DPAIR, 128, 4*FW] list.

    Pair d holds slices 2d, 2d+1; slice j of the pair occupies cols
    j*2*FW .. (j+1)*2*FW with target chunks (h-major 4x[128,512]) first,
    then pred chunks — 8 KB contiguous per SBUF partition per DMA."""
    n = B * C
    f8 = ml_dtypes.float8_e4m3
    x = np.empty((n, 128, 2 * FW), dtype=f8)
    t4 = target.reshape(n, NCHUNK, 128, W).astype(f8)
    p4 = pred.reshape(n, NCHUNK, 128, W).astype(f8)
    x[:, :, :FW] = t4.transpose(0, 2, 1, 3).reshape(n, 128, FW)
    x[:, :, FW:] = p4.transpose(0, 2, 1, 3).reshape(n, 128, FW)
    # pair consecutive slices side by side per partition
    x = x.reshape(n // 2, 2, 128, 2 * FW).transpose(0, 2, 1, 3).reshape(
        n // 2, 128, 4 * FW
    )
    x = np.ascontiguousarray(x)
    return [x[i * DPAIR : (i + 1) * DPAIR] for i in range(NCORES)]


def _combine(moments_per_core):
    loss_sum = 0.0
    n_valid = 0
    for O in moments_per_core:
        O = np.asarray(O, dtype=np.float64)
        for s in range(S):
            base = 5 * s
            T0 = O[base + 0, 0]
            S0 = O[base + 1, 0]
            Ty = O[base + 2, 0] * YSCALE
            Sy = O[base + 3, 0]
            Syy = O[base + 4, 0]
            Tx = O[base + 0, 1]
            Sx = O[base + 1, 1]
            Sxx = O[base + 1, 2]
            if T0 > 0:
                cy = Ty / T0
                cx = Tx / T0
                loss_sum += (
                    (Syy - 2.0 * cy * Sy + cy * cy * S0)
                    + (Sxx - 2.0 * cx * Sx + cx * cx * S0)
                ) / HW
                n_valid += 1
    if n_valid > 0:
        return np.float32(loss_sum / n_valid)
    return np.float32(0.0)


def kernel(pred, target):
    pred = np.ascontiguousarray(np.asarray(pred, dtype=np.float32))
    target = np.ascontiguousarray(np.asarray(target, dtype=np.float32))
    assert pred.shape == (B, C, H, W) and target.shape == (B, C, H, W)

    nc, (wmt, wmp, xcb, xc2b) = _get_built()
    xs = _pack_inputs(pred, target)

    in_maps = []
    for i in range(NCORES):
        in_maps.append(
            {"x": xs[i], "wmt": wmt, "wmp": wmp, "xcb": xcb, "xc2b": xc2b}
        )

    res = run_bass_kernel_spmd(nc, in_maps, list(range(NCORES)))
    outs = [res.results[i]["moments"] for i in range(NCORES)]
    return np.asarray(_combine(outs), dtype=np.float32)


if __name__ == "__main__":
    rng = np.random.default_rng(0)
    p = rng.standard_normal((B, C, H, W), dtype=np.float32)
    t = rng.random((B, C, H, W), dtype=np.float32)
    print(kernel(pred=p, target=t))


# revision 50
# speedup vs baseline: 1.0392x; 1.0392x over previous
"""Trainium2 Bass kernel for nn_ConcentrationLoss (raw-Bass SPMD version).

Math per (b, c) slice of pred/target [B,C,H,W]:
    mass = sum(t); cy = sum(t*y)/mass; cx = sum(t*x)/mass
    per_slice = mean(sigmoid(pred) * ((y-cy)^2 + (x-cx)^2))
    loss = mean(per_slice over slices with mass > 0)

Streaming moment sums per slice (centered coords y' = y-(H-1)/2,
x' = x-(W-1)/2):
    T0, Ty'        from target      S0, Sy', Sy'y'  from s = sigmoid(pred)
    Tx', Sx', Sx'x' (x-moments)

Host packs target+pred into one fp8-e4m3 tensor, two slices per 1 MiB DMA
([128, 8192]: slice j of the pair at cols j*4096, target chunks then pred
chunks, 8 KB contiguous per partition) — quarter the fp32 DMA traffic.
Quantization noise averages out over the 512x512 means (measured rel err
~2e-4 in fp64 simulation of this exact pipeline; gate is 2e-2).

On device: ScalarE sigmoids the pred half (fp8 in -> bf16 out); TensorE
contracts the h/partition axis — target matmuls in fp8 against y'/32-scaled
fp8 weight columns (descaled on host), pred matmuls in bf16 — accumulating
each slice's 5 moment rows [t, s, t*y'/32, s*y', s*y'^2] into one of 8 PSUM
banks. VectorE copies each bank to a base-0 scratch, GPSIMD SWDGE-DMAs the
scratch into a [100, 512] staging tile at partition offset 5*slice (DVE
cannot write at unaligned partition offsets), and VectorE runs one batched
stage2 (row sums, x'- and x'^2-weighted sums) per pass. Data-parallel on
batch across 8 cores; the scalar tail (centroid divides + masked mean) is
combined on the host.

Raw-Bass engine blocks with standalone wait_ge instructions (the toolchain
rejects instructions carrying more than one attached semaphore wait, which
rules out the Tile scheduler).
"""

import sys

for _p in ("/opt/trn_rl_repo",):
    if _p not in sys.path:
        sys.path.append(_p)

import numpy as np
import ml_dtypes

import concourse.bass as bass
from concourse import mybir
from concourse.bass_utils import run_bass_kernel_spmd

B, C, H, W = 16, 10, 512, 512
NCORES = 8
BPC = B // NCORES          # batches per core
S = BPC * C                # slices per core (20)
DPAIR = S // 2             # DMA pairs per core (2 slices / 1 MiB DMA)
NCHUNK = H // 128          # 4 h-chunks per slice
FW = NCHUNK * W            # 2048: free size of one packed half-tile
HW = float(H * W)
NROW = 5 * S               # 100 staging rows across all slices
YSCALE = 32.0              # target y' weight scale (power of 2, fp8-exact)

_CACHE = {}


def _moment_weights():
    """Returns (wmt fp8 [128, NCHUNK*5], wmp bf16 [128, NCHUNK*5]).
    Per chunk k: target lhsT cols k*5+0..4 -> psum rows [t, 0, t*y'/32, 0, 0];
    pred lhsT cols k*5+0..4 -> psum rows [0, s, 0, s*y', s*y'^2]."""
    # target weights in DoubleRow layout: DR-matmul j covers chunks 2j,2j+1;
    # k-tile i of DR-mm j lives at cols 32*j + 16*i + 0..4 (16 B middle step)
    wt = np.zeros((128, NCHUNK // 2 * 32), dtype=np.float64)
    wp = np.zeros((128, NCHUNK * 5), dtype=np.float64)
    for k in range(NCHUNK):
        yp = (np.arange(128, dtype=np.float64) + 128 * k) - (H - 1) / 2.0
        base = 32 * (k // 2) + 16 * (k % 2)
        wt[:, base + 0] = 1.0
        wt[:, base + 2] = yp / YSCALE
        wp[:, k * 5 + 1] = 1.0
        wp[:, k * 5 + 3] = yp
        wp[:, k * 5 + 4] = yp * yp
    return wt.astype(ml_dtypes.float8_e4m3), wp.astype(ml_dtypes.bfloat16)


def _build_nc(repeat=1, mode="full"):
    """repeat>1 re-runs the full pipeline (slice index s % S) for timing
    benchmarks — output is overwritten with identical values each pass.
    mode: 'full' | 'nostage2' (skip vector/gpsimd) | 'nomm' (dma+sigmoid
    only) | 'dmaonly' — timing probes with garbage output."""
    R = repeat
    do_sig = mode in ("full", "nostage2", "nomm")
    do_mm = mode in ("full", "nostage2")
    do_st2 = mode == "full"
    nc = bass.Bass("TRN2", target_bir_lowering=False, debug=False)
    f32, bf16, f8 = mybir.dt.float32, mybir.dt.bfloat16, mybir.dt.float8e4

    x_d = nc.dram_tensor("x", [DPAIR, 128, 4 * FW], f8, kind="ExternalInput")
    wmt_d = nc.dram_tensor("wmt", [128, NCHUNK // 2 * 32], f8, kind="ExternalInput")
    wmp_d = nc.dram_tensor("wmp", [128, NCHUNK * 5], bf16, kind="ExternalInput")
    xcb_d = nc.dram_tensor("xcb", [NROW, W], f32, kind="ExternalInput")
    xc2b_d = nc.dram_tensor("xc2b", [NROW, W], f32, kind="ExternalInput")
    out_d = nc.dram_tensor("moments", [NROW, 4], f32, kind="ExternalOutput")

    NB = 4                               # xb ring depth (DMA pairs in flight)
    PB = 4                               # pb ring depth (one tile per pair)
    xb = [nc.alloc_sbuf_tensor(f"xb{b}", [128, 4 * FW], f8) for b in range(NB)]
    pb = [nc.alloc_sbuf_tensor(f"pb{b}", [128, 2 * FW], bf16) for b in range(PB)]
    wtsb = nc.alloc_sbuf_tensor("wtsb", [128, NCHUNK // 2 * 32], f8)
    wpsb = nc.alloc_sbuf_tensor("wpsb", [128, NCHUNK * 5], bf16)
    xcsb = nc.alloc_sbuf_tensor("xcsb", [NROW, W], f32)
    xc2sb = nc.alloc_sbuf_tensor("xc2sb", [NROW, W], f32)
    sc = [nc.alloc_sbuf_tensor(f"sc{b}", [5, W], f32) for b in range(4)]
    stg = nc.alloc_sbuf_tensor("stg", [NROW, W], f32)
    t1 = nc.alloc_sbuf_tensor("t1", [NROW, W], f32)
    t2 = nc.alloc_sbuf_tensor("t2", [NROW, W], f32)
    O = nc.alloc_sbuf_tensor("O", [NROW, 4], f32)
    ps = [nc.alloc_psum_tensor(f"ps{b}", [5, W], f32) for b in range(8)]

    csem = nc.alloc_semaphore("csem")    # const DMAs
    xdma = [nc.alloc_semaphore(f"xdma{b}") for b in range(NB)]  # pair DMAs by slot
    asem = nc.alloc_semaphore("asem")    # sigmoid done (1 per slice)
    pe = nc.alloc_semaphore("pe")        # matmul group done (1 per slice)
    vcp = nc.alloc_semaphore("vcp")      # psum->scratch copy done (1 per slice)
    gsem = nc.alloc_semaphore("gsem")    # scratch->stg dma done (16 per slice)
    dst2 = nc.alloc_semaphore("dst2")    # stage2 done (1 per pass)
    osem = nc.alloc_semaphore("osem")    # out DMA

    def targ_ap(b, s, k):
        return xb[b][:, (s % 2) * 2 * FW + k * W : (s % 2) * 2 * FW + (k + 1) * W]

    def pred_ap(b, s):
        return xb[b][:, (s % 2) * 2 * FW + FW : (s % 2) * 2 * FW + 2 * FW]

    with nc.Block() as block:

        @block.sync
        def _(sync):
            # first payload pair leads; consts follow back-to-back (same
            # queue => FIFO completion). PE needs wmt/wmp only after the
            # first sigmoid (~6us in); DVE needs xcb/xc2b only at stage2.
            sync.dma_start(xb[0][:], x_d[0]).then_inc(xdma[0], 16)
            sync.dma_start(wtsb[:], wmt_d[:]).then_inc(csem, 16)
            sync.dma_start(wpsb[:], wmp_d[:]).then_inc(csem, 16)
            sync.dma_start(xcsb[:], xcb_d[:]).then_inc(csem, 16)
            sync.dma_start(xc2sb[:], xc2b_d[:]).then_inc(csem, 16)
            for d in range(1, R * DPAIR):
                b = d % NB
                if d >= NB:
                    # xb[b] consumed by both slices of pair d-NB
                    if do_mm:
                        sync.wait_ge(pe, 2 * (d - NB) + 2)
                    elif do_sig:
                        sync.wait_ge(asem, d - NB + 1)
                    sync.wait_ge(xdma[b], 16 * (d // NB))
                sync.dma_start(xb[b][:], x_d[d % DPAIR]).then_inc(xdma[b], 16)
            if do_st2:
                sync.wait_ge(dst2, R)
            elif do_mm:
                sync.wait_ge(pe, 2 * R * DPAIR)
            elif do_sig:
                sync.wait_ge(asem, R * DPAIR)
            else:
                for b in range(NB):
                    n_b = (R * DPAIR - b + NB - 1) // NB
                    if n_b > 0:
                        sync.wait_ge(xdma[b], 16 * n_b)
            sync.dma_start(out_d[:], O[:]).then_inc(osem, 16)
            sync.wait_ge(osem, 16)

        if do_sig:

            @block.scalar
            def _(scalar):
                for d in range(R * DPAIR):
                    b = d % NB
                    scalar.wait_ge(xdma[b], 16 * (d // NB + 1))
                    if d >= PB and do_mm:
                        scalar.wait_ge(pe, 2 * (d - PB) + 2)  # pb[d%PB] consumed
                    # both pred halves of the pair in one ACT instruction
                    # (3D in-AP strides over the two slice blocks)
                    scalar.activation(
                        pb[d % PB][:].rearrange("p (j f) -> p j f", j=2),
                        xb[b][:].rearrange("p (j f) -> p j f", j=2)[:, :, FW : 2 * FW],
                        mybir.ActivationFunctionType.Sigmoid,
                    ).then_inc(asem, 1)
                    if do_st2 and d == R * DPAIR - 1:
                        # last slice's staging copy via low-latency HWDGE
                        # instead of gpsimd SWDGE — shortens the tail
                        scalar.wait_ge(vcp, R * S)
                        scalar.dma_start(
                            stg[5 * (S - 1) : 5 * S, :], sc[(S - 1) % 4][:]
                        ).then_inc(gsem, 16)

        if do_mm:

            @block.tensor
            def _(tensor):
                tensor.wait_ge(csem, 32)          # wmt+wmp loaded (first consts)
                for s in range(2 * R * DPAIR):
                    d = s // 2
                    b = d % NB
                    p = ps[s % 8]
                    if s % 2 == 0:
                        tensor.wait_ge(xdma[b], 16 * (d // NB + 1))
                    if s >= 8 and do_st2:
                        tensor.wait_ge(vcp, s - 7)  # psum bank copied (slice s-8)
                    for j in range(NCHUNK // 2):
                        # DoubleRow: chunks 2j,2j+1 contracted in one pass
                        w_ap = wtsb[:].rearrange("p (j i c) -> p j i c", j=2, i=2)[
                            :, j, :, 0:5
                        ]
                        r_ap = xb[b][:].rearrange(
                            "p (sl j w) -> p sl j w", sl=2, j=8
                        )[:, s % 2, 2 * j : 2 * j + 2, :]
                        tensor.matmul(
                            p[:],
                            w_ap,
                            r_ap,
                            start=(j == 0),
                            stop=False,
                            perf_mode=mybir.MatmulPerfMode.DoubleRow,
                        )
                    tensor.wait_ge(asem, d + 1)   # pb ready for pred half
                    for k in range(NCHUNK):
                        mm = tensor.matmul(
                            p[:],
                            wpsb[:, k * 5 : k * 5 + 5],
                            pb[d % PB][:, (s % 2) * FW + k * W : (s % 2) * FW + (k + 1) * W],
                            start=False,
                            stop=(k == NCHUNK - 1),
                        )
                    mm.then_inc(pe, 1)

        if do_st2:

            @block.vector
            def _(vector):
                vector.wait_ge(csem, 64)
                for r in range(R):
                    for c in range(S):
                        s = r * S + c
                        vector.wait_ge(pe, s + 1)
                        if s >= 4:
                            vector.wait_ge(gsem, 16 * (s - 3))  # sc[s%4] drained
                        vector.tensor_copy(sc[s % 4][:], ps[s % 8][:]).then_inc(vcp, 1)
                    vector.wait_ge(gsem, 16 * S * (r + 1))      # stg fully packed
                    vector.reduce_sum(O[:, 0:1], stg[:], axis=mybir.AxisListType.X)
                    vector.tensor_mul(t1[:], stg[:], xcsb[:])
                    vector.reduce_sum(O[:, 1:2], t1[:], axis=mybir.AxisListType.X)
                    vector.tensor_mul(t2[:], stg[:], xc2sb[:])
                    vector.reduce_sum(
                        O[:, 2:3], t2[:], axis=mybir.AxisListType.X
                    ).then_inc(dst2, 1)

            @block.gpsimd
            def _(gpsimd):
                for s in range(R * S):
                    if s == R * S - 1:
                        continue          # final slice staged by scalar HWDGE
                    c = s % S
                    gpsimd.wait_ge(vcp, s + 1)
                    gpsimd.dma_start(stg[5 * c : 5 * c + 5, :], sc[s % 4][:]).then_inc(
                        gsem, 16
                    )

    return nc


def _host_inputs():
    xp = (np.arange(W, dtype=np.float64) - (W - 1) / 2.0).astype(np.float32)
    xcb = np.broadcast_to(xp, (NROW, W)).copy()
    xc2b = np.broadcast_to((xp * xp).astype(np.float32), (NROW, W)).copy()
    wmt, wmp = _moment_weights()
    return wmt, wmp, xcb, xc2b


def _get_built():
    if "nc" not in _CACHE:
        _CACHE["nc"] = _build_nc()
        _CACHE["consts"] = _host_inputs()
    return _CACHE["nc"], _CACHE["consts"]


def _pack_inputs(pred, target):
    """[B,C,H,W] fp32 pair -> per-core packed fp8 [DPAIR, 128, 4*FW] list.

    Pair d holds slices 2d, 2d+1; slice j of the pair occupies cols
    j*2*FW .. (j+1)*2*FW with target chunks (h-major 4x[128,512]) first,
    then pred chunks — 8 KB contiguous per SBUF partition per DMA."""
    n = B * C
    f8 = ml_dtypes.float8_e4m3
    x = np.empty((n, 128, 2 * FW), dtype=f8)
    t4 = target.reshape(n, NCHUNK, 128, W).astype(f8)
    p4 = pred.reshape(n, NCHUNK, 128, W).astype(f8)
    x[:, :, :FW] = t4.transpose(0, 2, 1, 3).reshape(n, 128, FW)
    x[:, :, FW:] = p4.transpose(0, 2, 1, 3).reshape(n, 128, FW)
    # pair consecutive slices side by side per partition
    x = x.reshape(n // 2, 2, 128, 2 * FW).transpose(0, 2, 1, 3).reshape(
        n // 2, 128, 4 * FW
    )
    x = np.ascontiguousarray(x)
    return [x[i * DPAIR : (i + 1) * DPAIR] for i in range(NCORES)]


def _combine(moments_per_core):
    loss_sum = 0.0
    n_valid = 0
    for O in moments_per_core:
        O = np.asarray(O, dtype=np.float64)
        for s in range(S):
            base = 5 * s
            T0 = O[base + 0, 0]
            S0 = O[base + 1, 0]
            Ty = O[base + 2, 0] * YSCALE
            Sy = O[base + 3, 0]
            Syy = O[base + 4, 0]
            Tx = O[base + 0, 1]
            Sx = O[base + 1, 1]
            Sxx = O[base + 1, 2]
            if T0 > 0:
                cy = Ty / T0
                cx = Tx / T0
                loss_sum += (
                    (Syy - 2.0 * cy * Sy + cy * cy * S0)
                    + (Sxx - 2.0 * cx * Sx + cx * cx * S0)
                ) / HW
                n_valid += 1
    if n_valid > 0:
        return np.float32(loss_sum / n_valid)
    return np.float32(0.0)


def kernel(pred, target):
    pred = np.ascontiguousarray(np.asarray(pred, dtype=np.float32))
    target = np.ascontiguousarray(np.asarray(target, dtype=np.float32))
    assert pred.shape == (B, C, H, W) and target.shape == (B, C, H, W)

    nc, (wmt, wmp, xcb, xc2b) = _get_built()
    xs = _pack_inputs(pred, target)

    in_maps = []
    for i in range(NCORES):
        in_maps.append(
            {"x": xs[i], "wmt": wmt, "wmp": wmp, "xcb": xcb, "xc2b": xc2b}
        )

    res = run_bass_kernel_spmd(nc, in_maps, list(range(NCORES)))
    outs = [res.results[i]["moments"] for i in range(NCORES)]
    return np.asarray(_combine(outs), dtype=np.float32)


if __name__ == "__main__":
    rng = np.random.default_rng(0)
    p = rng.standard_normal((B, C, H, W), dtype=np.float32)
    t = rng.random((B, C, H, W), dtype=np.float32)
    print(kernel(pred=p, target=t))
